# revision 1
# baseline (speedup 1.0000x reference)
"""Trainium2 Bass kernel for the GraphicalBranch GNN message-passing problem.

Math being computed (verified equivalent to the reference):
  - Per-sample graphs are fully connected WITH self-loops over the nc2=28
    pair-nodes, so segment_sum(x[src], dst) == broadcast of the per-sample
    row-sum S[b] = sum_r x[b, r, :].
  - The final key-matching gather h[rows] commutes with the row-wise linear
    layer, so we only run the W_self matmul on the 10 gathered rows per
    sample instead of all 28:
        out[b*10+k] = relu(xg[b*10+k] @ W_self + (S[b] @ W_nbr) + b)
  - rows are computed on host from slicing_tensor/object_pairs (pure index
    arithmetic) exactly as the reference's LUT does.

Sharding: data-parallel over samples; each of the 8 cores gets 128 samples
(3584 x-rows, 1280 output rows). Weights replicated.

Per-core device program (matmul operands bf16, f32 PSUM accumulate):
  1. S = G^T @ x on TensorE. x streams in 4 chunks of 896 rows (= exactly
     32 samples), so the same tiny one-hot block G[j][p, s] =
     ((j*128+p)//28 == s), s in [0,32), works for every chunk; chunk ch
     accumulates into PSUM partitions [32ch, 32ch+32) via
     tile_position=(0, 32ch). W_self matmuls for output tiles 0-3 are
     interleaved between chunks to fill PE gaps (their PSUM groups stay
     open until A is ready).
  2. Transpose S via 4 PE transposes -> S^T tiles (bf16).
  3. A = S @ W_nbr + b via 4 accumulating matmuls plus a K=1 ones-matmul
     that adds b to every row of the PSUM accumulator.
  4. Per output tile: 4 matmuls xg @ W_self (lhsT = xgT slices) + one
     expansion matmul E_t @ A (E[r, s] = 1 iff s == r//10) closing the
     same PSUM accumulation group; ReLU on ScalarE; stores in pairs.

All DRAM inputs are host-prelaid so every load is a plain contiguous
[128, F] DMA (one descriptor per partition). Loads split across the two
HWDGE rings in consumption order: the sync ring carries the big tensors
(x0, xgT, ws, x1..x3 — a single FIFO so arrival order is deterministic
and the PE never head-of-line blocks), the scalar ring the small/tail
set (g, wn, id, b, eT).
"""

import numpy as np
import ml_dtypes

# ---- problem constants (hardcoded; kernel.py must be self-contained) ----
B = 1024          # samples
NOBJ = 8          # objects per sample
NC2 = 28          # pair-nodes per sample
MAXR = 10         # relations per sample
D = 512           # feature dim
NCORES = 8
BL = B // NCORES          # 128 samples per core
RL = BL * NC2             # 3584 x-rows per core
ML = BL * MAXR            # 1280 output rows per core
KT = D // 128             # 4 contraction tiles
MT = ML // 128            # 10 output row tiles per core
RT = RL // 128            # 28 x row-tiles per core
XCH = 4                   # x chunks (896 rows = 32 samples each)
RJ = RT // XCH            # 7 row-tiles per chunk
SW = BL // XCH            # 32 samples per chunk

BF16 = ml_dtypes.bfloat16

_compiled = None


def _build_bass():
    import concourse.bacc as bacc
    import concourse.bass as bass
    import concourse.mybir as mybir
    from concourse import tile

    f32 = mybir.dt.float32
    bf16 = mybir.dt.bfloat16

    nc = bacc.Bacc("TRN2", target_bir_lowering=False, debug=False,
                   num_devices=NCORES)

    # all inputs prelaid on host: partition-major, contiguous free dim
    x_d = nc.dram_tensor("x", [XCH, 128, RJ * D], bf16, kind="ExternalInput")
    g_d = nc.dram_tensor("g", [128, RJ * SW], bf16, kind="ExternalInput")
    xgT_d = nc.dram_tensor("xgT", [128, KT * ML], bf16, kind="ExternalInput")
    ws_d = nc.dram_tensor("ws", [128, KT * D], bf16, kind="ExternalInput")
    wn_d = nc.dram_tensor("wn", [128, KT * D], bf16, kind="ExternalInput")
    eT_d = nc.dram_tensor("eT", [128, ML], bf16, kind="ExternalInput")
    b_d = nc.dram_tensor("bias", [1, D], bf16, kind="ExternalInput")
    id_d = nc.dram_tensor("ident", [128, 128], bf16, kind="ExternalInput")
    out_d = nc.dram_tensor("out", [ML, D], f32, kind="ExternalOutput")

    with tile.TileContext(nc) as tc:
        with (
            tc.tile_pool(name="const", bufs=1) as cpool,
            tc.tile_pool(name="x", bufs=4) as xpool,
            tc.tile_pool(name="outp", bufs=3) as opool,
            tc.tile_pool(name="psum", bufs=4, space=bass.MemorySpace.PSUM) as ppool,
            tc.tile_pool(name="psumS", bufs=1, space=bass.MemorySpace.PSUM) as pspool,
            tc.tile_pool(name="psumT", bufs=2, space=bass.MemorySpace.PSUM) as ptpool,
            tc.tile_pool(name="psumA", bufs=1, space=bass.MemorySpace.PSUM) as papool,
        ):
            # ---- loads: sync ring carries x0, xgT, ws, x1..x3 in
            # ---- consumption order; scalar ring carries the small/tail set
            g_sb = cpool.tile([128, RJ, SW], bf16)
            nc.scalar.dma_start(g_sb[:], g_d.rearrange("p (j s) -> p j s", s=SW))
            wn_sb = cpool.tile([128, KT, D], bf16)
            nc.scalar.dma_start(wn_sb[:], wn_d.rearrange("p (t n) -> p t n", n=D))
            id_sb = cpool.tile([128, 128], bf16)
            nc.scalar.dma_start(id_sb[:], id_d[:, :])
            b_sb = cpool.tile([1, D], bf16)
            nc.scalar.dma_start(b_sb[:], b_d[:, :])
            eT_sb = cpool.tile([128, ML], bf16)
            nc.scalar.dma_start(eT_sb[:], eT_d[:, :])
            ones_sb = cpool.tile([1, 128], bf16)
            nc.gpsimd.memset(ones_sb[:], 1.0)

            # ---- S accumulation, interleaved with early W_self groups ----
            psS = pspool.tile([128, D], f32)
            main_ps = {}

            def open_main_group(t):
                ps = ppool.tile([128, D], f32, tag="ps")
                for kt in range(KT):
                    nc.tensor.matmul(
                        ps[:],
                        xgT_sb[:, kt, t * 128:(t + 1) * 128],
                        ws_sb[:, kt, :],
                        start=(kt == 0), stop=False,
                    )
                main_ps[t] = ps

            for ch in range(XCH):
                xch = xpool.tile([128, RJ, D], bf16, tag="x")
                nc.sync.dma_start(xch[:], x_d[ch].rearrange("p (j d) -> p j d", d=D))
                if ch == 0:
                    xgT_sb = cpool.tile([128, KT, ML], bf16)
                    nc.sync.dma_start(
                        xgT_sb[:], xgT_d.rearrange("p (t m) -> p t m", m=ML))
                    ws_sb = cpool.tile([128, KT, D], bf16)
                    nc.sync.dma_start(
                        ws_sb[:], ws_d.rearrange("p (t n) -> p t n", n=D))
                for j in range(RJ):
                    nc.tensor.matmul(psS[ch * SW:(ch + 1) * SW, :],
                                     g_sb[:, j, :], xch[:, j, :],
                                     start=(j == 0), stop=(j == RJ - 1),
                                     tile_position=(0, ch * SW))
                open_main_group(ch)   # fill PE while next chunk streams

            s_nat = cpool.tile([128, D], bf16)
            nc.scalar.copy(s_nat[:], psS[:])

            # ---- transpose S -> S^T (bf16) ----
            s_bf = cpool.tile([128, KT, BL], bf16)
            for dt in range(KT):
                psT = ptpool.tile([128, BL], bf16, tag="psT")
                nc.tensor.transpose(psT[:], s_nat[:, dt * 128:(dt + 1) * 128],
                                    id_sb[:])
                nc.vector.tensor_copy(s_bf[:, dt, :], psT[:])

            # ---- A = S @ W_nbr + b (bias via K=1 ones matmul) ----
            psA = papool.tile([128, D], f32)
            for kt in range(KT):
                nc.tensor.matmul(psA[:], s_bf[:, kt, :], wn_sb[:, kt, :],
                                 start=(kt == 0), stop=False)
            nc.tensor.matmul(psA[:], ones_sb[:], b_sb[:],
                             start=False, stop=True)
            a_bf = cpool.tile([128, D], bf16)
            nc.vector.tensor_copy(a_bf[:], psA[:])

            # ---- close groups / remaining tiles; stores in pairs ----
            out_r = out_d.rearrange("(t u p) n -> t p u n", p=128, u=2)
            ot = None
            for t in range(MT):
                if t not in main_ps:
                    open_main_group(t)
                ps = main_ps.pop(t)
                nc.tensor.matmul(ps[:], eT_sb[:, t * 128:(t + 1) * 128],
                                 a_bf[:], start=False, stop=True)
                if t % 2 == 0:
                    ot = opool.tile([128, 2, D], f32, tag="ot")
                nc.scalar.activation(ot[:, t % 2, :], ps[:],
                                     mybir.ActivationFunctionType.Relu)
                if t % 2 == 1:
                    nc.sync.dma_start(out_r[t // 2], ot[:])

    nc.compile()
    return nc


def _get_compiled():
    global _compiled
    if _compiled is None:
        _compiled = _build_bass()
    return _compiled


def _host_prep(inputs):
    """Shard + preprocess on host. Returns per-core input maps."""
    x = np.asarray(inputs["spatial_branch_feature_map"], dtype=np.float32)
    W_self = np.asarray(inputs["W_self"], dtype=np.float32)
    W_nbr = np.asarray(inputs["W_nbr"], dtype=np.float32)
    b = np.asarray(inputs["b"], dtype=np.float32)
    st = np.asarray(inputs["slicing_tensor"])
    op = np.asarray(inputs["object_pairs"])

    N = x.shape[0]
    n = NOBJ
    # exact replication of the reference's LUT-based row computation
    keys = st[:, 0].astype(np.int64) * (n * n) + st[:, 1].astype(np.int64) * n \
        + st[:, 2].astype(np.int64)
    lut = np.zeros(B * n * n, dtype=np.int64)
    lut[keys] = np.arange(N, dtype=np.int64)
    pmin = np.minimum(op[..., 0], op[..., 1]).astype(np.int64)
    pmax = np.maximum(op[..., 0], op[..., 1]).astype(np.int64)
    rel_keys = (np.arange(B, dtype=np.int64)[:, None] * (n * n)
                + pmin * n + pmax).reshape(-1)
    rows = lut[rel_keys]                      # [B*MAXR] global row index

    xg = x[rows]                              # [B*MAXR, D]
    # x: [NCORES, XCH, 128, RJ*D]; sbuf[p, j, :] = x_core[ch*896 + j*128 + p]
    x_bf = np.ascontiguousarray(
        x.astype(BF16).reshape(NCORES, XCH, RJ, 128, D)
        .transpose(0, 1, 3, 2, 4).reshape(NCORES, XCH, 128, RJ * D))
    # xgT: [NCORES, 128, KT*ML]; sbuf[p, kt, m] = xg_core[m, kt*128+p]
    xgT = np.ascontiguousarray(
        xg.astype(BF16).reshape(NCORES, ML, KT, 128)
        .transpose(0, 3, 2, 1).reshape(NCORES, 128, KT * ML))

    def wlay(W):  # [D, D] -> [128, KT*D]: sbuf[p, kt, n] = W[kt*128+p, n]
        return np.ascontiguousarray(
            W.astype(BF16).reshape(KT, 128, D).transpose(1, 0, 2)
            .reshape(128, KT * D))

    ws = wlay(W_self)
    wn = wlay(W_nbr)
    eT = (np.arange(ML)[None, :] // MAXR
          == np.arange(128)[:, None]).astype(BF16)   # [128, ML]
    # shared one-hot block: g[p, j*SW + s] = ((j*128 + p)//NC2 == s)
    jj = np.arange(RJ * 128)
    g = (jj[:, None] // NC2 == np.arange(SW)[None, :]).astype(BF16)
    g = np.ascontiguousarray(
        g.reshape(RJ, 128, SW).transpose(1, 0, 2).reshape(128, RJ * SW))
    bias = b.astype(BF16).reshape(1, D)
    ident = np.eye(128, dtype=BF16)

    in_maps = []
    for c in range(NCORES):
        in_maps.append({
            "x": x_bf[c], "xgT": xgT[c], "g": g,
            "ws": ws, "wn": wn, "eT": eT, "bias": bias, "ident": ident,
        })
    return in_maps


def run(inputs, trace=False):
    """Returns (full_output, BassKernelResults)."""
    from concourse.bass_utils import run_bass_kernel_spmd

    nc = _get_compiled()
    in_maps = _host_prep(inputs)
    res = run_bass_kernel_spmd(nc, in_maps, core_ids=list(range(NCORES)),
                               trace=trace)
    out = np.concatenate([r["out"] for r in res.results], axis=0)
    return out, res


def kernel(**inputs) -> np.ndarray:
    out, _ = run(inputs, trace=False)
    return out



# revision 4
# speedup vs baseline: 1.1441x; 1.1441x over previous
"""Trainium2 Bass kernel for the GraphicalBranch GNN message-passing problem.

Math (equivalent to the reference):
  - Per-sample graphs are fully connected WITH self-loops over the nc2=28
    pair-nodes, so segment_sum(x[src], dst) == broadcast of the per-sample
    row-sum S[b] = sum_r x[b, r, :].
  - The final key-matching gather h[rows] commutes with the row-wise linear
    layer, so only the 10 gathered rows per sample are pushed through W_self:
        out[row] = relu(xg[row] @ W_self + (S[b(row)] @ W_nbr + b))
  - rows are computed on host from slicing_tensor/object_pairs (pure index
    arithmetic), exactly as the reference's LUT does.

Device-side structure (per core: 128 samples, 3584 x-rows, 1280 out rows):
  - Output rows are regrouped into 10 tiles of 128 rows keyed (t = h*5+rp):
    partition m of tile t holds relation r=2*rp+m//64 of sample s=64h+m%64.
    With this layout the neighbor term A[s] is PARTITION-ALIGNED for the
    whole tile, so the A-add is a single DVE tensor_tensor add (the baseline
    needed a one-hot expansion matmul per tile).
  - Samples are processed in 2 halves of 64 (= 2 chunks of 32) that pipeline
    through one bufs=2 PSUM pool: psS (aggregation) -> psTr (transposes) ->
    psA (A matmul). Each PSUM bank is written, read once, then freed, so no
    PSUM bank is ever read while a later matmul still writes it.
  - Aggregation: one-hot G as lhsT, x as rhs; the two 32-sample chunks of a
    half run as column-tiled concurrent matmuls (tile_position 0/32).
  - Output is stored in bf16 (host upcasts to f32); rel-err budget is ~2e-2
    and bf16 rounding of the output costs < 0.2% absmax.
"""

import numpy as np
import ml_dtypes

# ---- problem constants (hardcoded; kernel.py must be self-contained) ----
B = 1024          # samples
NOBJ = 8          # objects per sample
NC2 = 28          # pair-nodes per sample
MAXR = 10         # relations per sample
D = 512           # feature dim
NCORES = 8
BL = B // NCORES          # 128 samples per core
RL = BL * NC2             # 3584 x-rows per core
ML = BL * MAXR            # 1280 output rows per core
KT = D // 128             # 4 contraction tiles
NH = 2                    # sample halves per core
HS = BL // NH             # 64 samples per half
NCH = 2                   # chunks per half
CS = HS // NCH            # 32 samples per chunk
NJ = 7                    # 128-row tiles per chunk (896 rows)
NT = 10                   # output tiles per core
NPRE = 5                  # xgT tiles prefetched before x

BF16 = ml_dtypes.bfloat16

_compiled = None


def _build_bass():
    import concourse.bacc as bacc
    import concourse.bass as bass
    import concourse.mybir as mybir
    from concourse import tile

    f32 = mybir.dt.float32
    bf16 = mybir.dt.bfloat16

    nc = bacc.Bacc("TRN2", target_bir_lowering=False, debug=False,
                   num_devices=NCORES)

    # x tiles per half, ordered (j, u): tile 2j+u = rows j*128..j*128+128 of
    # chunk u, so the two chunks' j-th tiles sit adjacent for col-tiled agg.
    x_d = nc.dram_tensor("x", [NH, 128, 14, D], bf16, kind="ExternalInput")
    g_d = nc.dram_tensor("g", [128, NJ, CS], bf16, kind="ExternalInput")
    xgT_d = nc.dram_tensor("xgT", [128, NT, KT, 128], bf16,
                           kind="ExternalInput")
    ws_d = nc.dram_tensor("ws", [128, KT, D], bf16, kind="ExternalInput")
    wn_d = nc.dram_tensor("wn", [128, KT, D], bf16, kind="ExternalInput")
    b_d = nc.dram_tensor("bias", [1, D], bf16, kind="ExternalInput")
    id_d = nc.dram_tensor("ident", [HS, HS], bf16, kind="ExternalInput")
    out_d = nc.dram_tensor("out", [NT, 128, D], bf16, kind="ExternalOutput")

    with tile.TileContext(nc) as tc:
        with (
            tc.tile_pool(name="const", bufs=1) as cpool,
            tc.tile_pool(name="tmp", bufs=2) as tpool,
            tc.tile_pool(name="outp", bufs=3) as opool,
            tc.tile_pool(name="chain", bufs=2,
                         space=bass.MemorySpace.PSUM) as chain,
            tc.tile_pool(name="mains", bufs=6,
                         space=bass.MemorySpace.PSUM) as mains,
        ):
            # ---- loads: sync ring in consumption order ----
            g_sb = cpool.tile([128, NJ, CS], bf16)
            nc.sync.dma_start(g_sb[:], g_d[:, :, :])
            ws_sb = cpool.tile([128, KT, D], bf16)
            nc.sync.dma_start(ws_sb[:], ws_d[:, :, :])
            wn_sb = cpool.tile([128, KT, D], bf16)
            nc.sync.dma_start(wn_sb[:], wn_d[:, :, :])
            xgT_a = cpool.tile([128, NPRE, KT, 128], bf16)
            nc.sync.dma_start(xgT_a[:], xgT_d[:, 0:NPRE, :, :])
            x_sb = []
            for h in range(NH):
                xa = cpool.tile([128, 8, D], bf16)
                nc.sync.dma_start(xa[:], x_d[h][:, 0:8, :])
                xb = cpool.tile([128, 6, D], bf16)
                nc.sync.dma_start(xb[:], x_d[h][:, 8:14, :])
                x_sb.append((xa, xb))
            xgT_b = cpool.tile([128, NT - NPRE, KT, 128], bf16)
            nc.sync.dma_start(xgT_b[:], xgT_d[:, NPRE:NT, :, :])

            # ---- small loads on the scalar ring ----
            id_sb = cpool.tile([HS, HS], bf16)
            nc.scalar.dma_start(id_sb[:], id_d[:, :])
            b_sb = cpool.tile([1, D], bf16)
            nc.scalar.dma_start(b_sb[:], b_d[:, :])
            ones_sb = cpool.tile([1, 128], bf16)
            nc.gpsimd.memset(ones_sb[:], 1.0)

            def xtile(h, idx):
                xa, xb = x_sb[h]
                return xa[:, idx, :] if idx < 8 else xb[:, idx - 8, :]

            def xgt(t, kt):
                if t < NPRE:
                    return xgT_a[:, t, kt, :]
                return xgT_b[:, t - NPRE, kt, :]

            def open_group(t):
                ps = mains.tile([128, D], f32, tag="ps")
                for kt in range(KT):
                    nc.tensor.matmul(ps[:], xgt(t, kt), ws_sb[:, kt, :],
                                     start=(kt == 0), stop=(kt == KT - 1))
                return ps

            def evac(t, ps, a2):
                tmp = tpool.tile([128, D], bf16, tag="tmp")
                nc.vector.tensor_tensor(tmp[:], ps[:], a2[:],
                                        op=mybir.AluOpType.add)
                ot = opool.tile([128, D], bf16, tag="ot")
                nc.scalar.activation(ot[:], tmp[:],
                                     mybir.ActivationFunctionType.Relu)
                nc.sync.dma_start(out_d[t], ot[:])

            a_of = [None, None]
            for h in range(NH):
                # -- aggregation: S over this half's 64 samples --
                # one PSUM bank per chunk: concurrent accumulation groups
                # must not share a bank (start=True zeroes per-bank state)
                psS = []
                for u in range(NCH):
                    psS_u = chain.tile([128, D], f32, tag="chain",
                                       name=f"psS_{h}_{u}")
                    psS.append(psS_u)
                for j in range(NJ):
                    for u in range(NCH):
                        nc.tensor.matmul(
                            psS[u][32 * u:32 * u + 32, :],
                            g_sb[:, j, :], xtile(h, 2 * j + u),
                            start=(j == 0), stop=(j == NJ - 1),
                            tile_position=(0, 32 * u),
                        )
                s_nat = cpool.tile([HS, D], bf16)
                for u in range(NCH):
                    nc.vector.tensor_copy(s_nat[32 * u:32 * u + 32, :],
                                          psS[u][32 * u:32 * u + 32, :])

                # -- S^T via PE transposes --
                psTr = chain.tile([128, KT, HS], bf16, tag="chain")
                for dt in range(KT):
                    nc.tensor.transpose(psTr[:, dt, :],
                                        s_nat[:, dt * 128:(dt + 1) * 128],
                                        id_sb[:])
                # column-doubled S^T so A lands on all 128 psum partitions
                sT2 = cpool.tile([128, KT, 128], bf16)
                nc.vector.tensor_copy(sT2[:, :, 0:HS], psTr[:])
                nc.vector.tensor_copy(sT2[:, :, HS:128], psTr[:])

                # -- A = S @ W_nbr + b (bias via K=1 ones matmul) --
                psA = chain.tile([128, D], f32, tag="chain")
                for kt in range(KT):
                    nc.tensor.matmul(psA[:], sT2[:, kt, :], wn_sb[:, kt, :],
                                     start=(kt == 0), stop=False)
                nc.tensor.matmul(psA[:], ones_sb[:1, :], b_sb[:],
                                 start=False, stop=True)
                a2 = cpool.tile([128, D], f32)
                nc.vector.tensor_copy(a2[:], psA[:])
                a_of[h] = a2

                # -- W_self groups of this half --
                for rp in range(5):
                    t = h * 5 + rp
                    ps = open_group(t)
                    evac(t, ps, a2)

    nc.compile()
    return nc


def _get_compiled():
    global _compiled
    if _compiled is None:
        _compiled = _build_bass()
    return _compiled


def _rowl_table():
    """row_local[t, m]: xg row (s*10+r) held by partition m of out tile t."""
    t = np.arange(NT)[:, None]
    m = np.arange(128)[None, :]
    h, rp = t // 5, t % 5
    r = 2 * rp + m // HS
    s = HS * h + m % HS
    return (s * MAXR + r).astype(np.int64)


def _host_prep(inputs):
    """Shard + preprocess on host. Returns per-core input maps."""
    x = np.asarray(inputs["spatial_branch_feature_map"], dtype=np.float32)
    W_self = np.asarray(inputs["W_self"], dtype=np.float32)
    W_nbr = np.asarray(inputs["W_nbr"], dtype=np.float32)
    b = np.asarray(inputs["b"], dtype=np.float32)
    st = np.asarray(inputs["slicing_tensor"])
    op = np.asarray(inputs["object_pairs"])

    N = x.shape[0]
    n = NOBJ
    # exact replication of the reference's LUT-based row computation
    keys = st[:, 0].astype(np.int64) * (n * n) + st[:, 1].astype(np.int64) * n \
        + st[:, 2].astype(np.int64)
    lut = np.zeros(B * n * n, dtype=np.int64)
    lut[keys] = np.arange(N, dtype=np.int64)
    pmin = np.minimum(op[..., 0], op[..., 1]).astype(np.int64)
    pmax = np.maximum(op[..., 0], op[..., 1]).astype(np.int64)
    rel_keys = (np.arange(B, dtype=np.int64)[:, None] * (n * n)
                + pmin * n + pmax).reshape(-1)
    rows = lut[rel_keys]                      # [B*MAXR] global row index

    rowl = _rowl_table()                      # [NT, 128]

    # x: [NCORES, NH, 128, 14, D]; tile 2j+u = rows j*128.. of chunk 2h+u
    x_bf = (x.astype(BF16)
            .reshape(NCORES, NH, NCH, NJ, 128, D)      # [c, h, u, j, p, d]
            .transpose(0, 1, 4, 3, 2, 5)               # [c, h, p, j, u, d]
            .reshape(NCORES, NH, 128, 14, D))
    x_bf = np.ascontiguousarray(x_bf)

    # xgT: [NCORES, 128, NT, KT, 128]; [p, t, kt, m] = xg[rowl[t,m], kt*128+p]
    xg = x[rows].astype(BF16).reshape(NCORES, ML, D)
    xgT = np.empty((NCORES, 128, NT, KT, 128), dtype=BF16)
    for c in range(NCORES):
        sel = xg[c][rowl.ravel()]             # [NT*128, D]
        xgT[c] = (sel.reshape(NT, 128, KT, 128)        # [t, m, kt, p]
                  .transpose(3, 0, 2, 1))              # [p, t, kt, m]
    xgT = np.ascontiguousarray(xgT)

    def wlay(W):  # [D, D] -> [128, KT, D]: [p, kt, n] = W[kt*128+p, n]
        return np.ascontiguousarray(
            W.astype(BF16).reshape(KT, 128, D).transpose(1, 0, 2))

    ws = wlay(W_self)
    wn = wlay(W_nbr)
    # one-hot agg block: g[p, j, s] = ((j*128 + p)//NC2 == s), s in [0, 32)
    jj = np.arange(NJ * 128)
    g = (jj[:, None] // NC2 == np.arange(CS)[None, :]).astype(BF16)
    g = np.ascontiguousarray(
        g.reshape(NJ, 128, CS).transpose(1, 0, 2))
    bias = b.astype(BF16).reshape(1, D)
    ident = np.eye(HS, dtype=BF16)

    in_maps = []
    for c in range(NCORES):
        in_maps.append({
            "x": x_bf[c], "xgT": xgT[c], "g": g,
            "ws": ws, "wn": wn, "bias": bias, "ident": ident,
        })
    return in_maps


def _unpermute(out_cores):
    """[NCORES][NT, 128, D] bf16 -> [B*MAXR, D] f32 in reference order."""
    rowl = _rowl_table().ravel()
    out = np.empty((NCORES * ML, D), dtype=np.float32)
    for c in range(NCORES):
        oc = np.asarray(out_cores[c]).reshape(NT * 128, D)
        out[c * ML + rowl] = oc.astype(np.float32)
    return out


def run(inputs, trace=False):
    """Returns (full_output, BassKernelResults)."""
    from concourse.bass_utils import run_bass_kernel_spmd

    nc = _get_compiled()
    in_maps = _host_prep(inputs)
    res = run_bass_kernel_spmd(nc, in_maps, core_ids=list(range(NCORES)),
                               trace=trace)
    out = _unpermute([r["out"] for r in res.results])
    return out, res


def kernel(**inputs) -> np.ndarray:
    out, _ = run(inputs, trace=False)
    return out
